# revision 1
# baseline (speedup 1.0000x reference)
"""MatchLSTM Trainium2 kernel: data-parallel over batch (8 cores, 1 batch elem each).

Per-core program (B=1): embedding gather -> transposed-input projections ->
q-GRU (64 steps) -> ctx-GRU (T steps) -> match recurrence (T steps).
Row-layout gate math on partition 0, column-layout hidden state (via PE
transpose) feeding bf16 matmuls; fp32 PSUM accumulation and fp32 nonlinearities.
"""
import math
from contextlib import ExitStack

import numpy as np
import ml_dtypes

import concourse.bacc as bacc
import concourse.bass as bass
import concourse.mybir as mybir
import concourse.tile as tile
from concourse.bass_utils import run_bass_kernel_spmd

F32 = mybir.dt.float32
BF16 = mybir.dt.bfloat16
I32 = mybir.dt.int32
AF = mybir.ActivationFunctionType
OP = mybir.AluOpType
BF = ml_dtypes.bfloat16

H = 150
D = 300
J = 64
V = 100000


def _chunks(n, c=128):
    return [min(c, n - i) for i in range(0, n, c)]


def build(T=400):
    NT = math.ceil(T / 128)
    tsz = _chunks(T)  # t-chunk sizes for XPc/ZX/WHP tiles

    nc = bacc.Bacc("TRN2", target_bir_lowering=False, debug=False, num_devices=8)

    # ---- DRAM inputs ----
    dram = {}

    def din(name, shape, dt):
        dram[name] = nc.dram_tensor(name, list(shape), dt, kind="ExternalInput")
        return dram[name]

    E_d = din("E", [V, D], F32)
    cidx_d = din("ctx_idx", [128, NT], I32)
    qidx_d = din("q_idx", [J, 1], I32)
    Ifp_d = din("Ifp", [128, 128], F32)
    Ibf_d = din("Ibf", [128, 128], BF16)
    ones64_d = din("ones64", [2, 64], BF16)
    wbc_d = din("w_bcast", [J, H], BF16)
    onesr_d = din("ones_row", [1, 512], BF16)
    wnames = []
    for g in ("q", "c"):
        wnames += [(f"WihT_{g}_0", (128, 450)), (f"WihT_{g}_1", (128, 450)),
                   (f"WihT_{g}_2", (45, 450)),
                   (f"WhhT_{g}_0", (128, 450)), (f"WhhT_{g}_1", (23, 450))]
    wnames += [("WhhT_m_0", (128, 450)), ("WhhT_m_1", (23, 450)),
               ("W2T_0", (128, 450)), ("W2T_1", (23, 450)),
               ("WcT_0", (128, 450)), ("WcT_1", (23, 450)),
               ("Wr_0", (128, H)), ("Wr_1", (23, H)),
               ("Wp_0", (128, H)), ("Wp_1", (23, H)),
               ("Wq_0", (128, H)), ("Wq_1", (23, H))]
    for n, s in wnames:
        din(n, s, BF16)
    hr_d = nc.dram_tensor("hr", [T + 1, H], F32, kind="ExternalOutput")

    with tile.TileContext(nc) as tc, ExitStack() as st:
        sb = st.enter_context(tc.tile_pool(name="sb", bufs=1))

        def sbt(name, shape, dt):
            return sb.tile(list(shape), dt, tag=name, name=name)

        # ---- persistent SBUF tiles ----
        W = {n: sbt(n, s, BF16) for n, s in wnames}
        Ifp = sbt("Ifp", (128, 128), F32)
        Ibf = sbt("Ibf", (128, 128), BF16)
        ones64 = sbt("ones64", (2, 64), BF16)
        wbc = sbt("wbc", (J, H), BF16)
        cidx = sbt("cidx", (128, NT), I32)
        qidx = sbt("qidx", (J, 1), I32)
        ec = [sbt(f"ec{g}", (128, D), F32) for g in range(NT)]
        eq = sbt("eq", (J, D), F32)
        ecT = [sbt("ecT0", (128, T), BF16), sbt("ecT1", (128, T), BF16),
               sbt("ecT2", (45, T), BF16)]
        eqT = [sbt("eqT0", (128, J), BF16), sbt("eqT1", (128, J), BF16),
               sbt("eqT2", (45, J), BF16)]
        XPc = [sbt(f"XPc{g}", (tsz[g], 450), BF16) for g in range(NT)]
        XPq = sbt("XPq", (J, 450), BF16)
        HqC = [sbt("HqC0", (128, J + 1), BF16), sbt("HqC1", (23, J + 1), BF16)]
        HcC = [sbt("HcC0", (128, T + 1), BF16), sbt("HcC1", (23, T + 1), BF16)]
        ZX = [sbt(f"ZX{g}", (tsz[g], 450), BF16) for g in range(NT)]
        WHP = [sbt(f"WHP{g}", (tsz[g], H), BF16) for g in range(NT)]
        whq = sbt("whq", (J, H), BF16)
        HqR = sbt("HqR", (J, H), BF16)
        G_sb = sbt("G_sb", (J, H), BF16)
        Gscr = sbt("Gscr", (J, H), BF16)
        attn_f = sbt("attn_f", (J, 1), F32)
        attn_b = sbt("attn_b", (J, 1), BF16)
        s_row = sbt("s_row", (2, H), BF16)
        aq_a = sbt("aq_a", (128, 1), BF16)
        aq_b = sbt("aq_b", (23, 1), BF16)
        hm = [sbt("hm0", (128, 1), BF16), sbt("hm1", (23, 1), BF16)]
        zrow = sbt("zrow", (1, H), F32)
        # per-GRU row scratch
        rows = {}
        for g in ("q", "c", "m"):
            rows[g] = dict(
                hrow=sbt(f"hrow_{g}", (1, H), F32),
                sig=sbt(f"sig_{g}", (1, 300), F32),
                rhn=sbt(f"rhn_{g}", (1, H), F32),
                narg=sbt(f"narg_{g}", (1, H), F32),
                nn=sbt(f"nn_{g}", (1, H), F32),
                dd=sbt(f"dd_{g}", (1, H), F32),
                ee=sbt(f"ee_{g}", (1, H), F32),
            )

        # ---- load constants / weights ----
        for n, _ in wnames:
            nc.sync.dma_start(W[n][:], dram[n].ap())
        nc.sync.dma_start(Ifp[:], Ifp_d.ap())
        nc.sync.dma_start(Ibf[:], Ibf_d.ap())
        nc.sync.dma_start(ones64[:], ones64_d.ap())
        nc.sync.dma_start(wbc[:], wbc_d.ap())
        nc.sync.dma_start(cidx[:], cidx_d.ap())
        nc.sync.dma_start(qidx[:], qidx_d.ap())

        # ---- init state ----
        for hc, ncol in ((HqC, J + 1), (HcC, T + 1)):
            nc.vector.memset(hc[0][:, 0:1], 0.0)
            nc.vector.memset(hc[1][0:22, 0:1], 0.0)
            nc.sync.dma_start(hc[1][22:23, 0:ncol], onesr_d.ap()[0:1, 0:ncol])
        nc.vector.memset(hm[0][:], 0.0)
        nc.vector.memset(hm[1][0:22, :], 0.0)
        nc.sync.dma_start(hm[1][22:23, 0:1], onesr_d.ap()[0:1, 0:1])
        nc.sync.dma_start(ecT[2][44:45, 0:T], onesr_d.ap()[0:1, 0:T])
        nc.sync.dma_start(eqT[2][44:45, 0:J], onesr_d.ap()[0:1, 0:J])
        nc.sync.dma_start(aq_b[22:23, 0:1], onesr_d.ap()[0:1, 0:1])
        nc.vector.memset(zrow[:], 0.0)
        nc.vector.memset(s_row[:], 0.0)
        for g in ("q", "c", "m"):
            nc.vector.memset(rows[g]["hrow"][:], 0.0)
        nc.sync.dma_start(hr_d.ap()[0:1, :], zrow[0:1, :])

        # ---- gathers ----
        for g in range(NT):
            nc.gpsimd.indirect_dma_start(
                out=ec[g][:], out_offset=None, in_=E_d.ap(),
                in_offset=bass.IndirectOffsetOnAxis(ap=cidx[:, g:g + 1], axis=0))
        nc.gpsimd.indirect_dma_start(
            out=eq[:], out_offset=None, in_=E_d.ap(),
            in_offset=bass.IndirectOffsetOnAxis(ap=qidx[:, 0:1], axis=0))

        dch = [(0, 128), (128, 128), (256, 44)]  # d-chunks of embedding dim

        # ---- preamble: transposes + input projections ----
        with tc.tile_pool(name="pre_ps", bufs=2, space="PSUM") as pps, \
             tc.tile_pool(name="xp_ps", bufs=2, space="PSUM") as xps:
            for g in range(NT):
                toff = 128 * g
                for k, (doff, dsz) in enumerate(dch):
                    tp = pps.tile([128, 128], F32, tag="tp", name="tp")
                    nc.tensor.transpose(tp[0:dsz, 0:tsz[g]],
                                        ec[g][0:tsz[g], doff:doff + dsz],
                                        Ifp[0:tsz[g], 0:tsz[g]])
                    nc.scalar.copy(ecT[k][0:dsz, toff:toff + tsz[g]],
                                   tp[0:dsz, 0:tsz[g]])
            for k, (doff, dsz) in enumerate(dch):
                tp = pps.tile([128, 128], F32, tag="tp", name="tp")
                nc.tensor.transpose(tp[0:dsz, 0:J], eq[0:J, doff:doff + dsz],
                                    Ifp[0:J, 0:J])
                nc.scalar.copy(eqT[k][0:dsz, 0:J], tp[0:dsz, 0:J])
            # XPq = [eq;1] @ [WihT_q; bih]
            xq = xps.tile([J, 450], F32, tag="xp", name="xp")
            for k in range(3):
                ksz = [128, 128, 45][k]
                nc.tensor.matmul(xq[0:J, :], eqT[k][0:ksz, 0:J],
                                 W[f"WihT_q_{k}"][0:ksz, :],
                                 start=(k == 0), stop=(k == 2))
            nc.vector.tensor_copy(XPq[:], xq[0:J, :])
            for g in range(NT):
                xc = xps.tile([128, 450], F32, tag="xp", name="xp")
                for k in range(3):
                    ksz = [128, 128, 45][k]
                    nc.tensor.matmul(xc[0:tsz[g], :],
                                     ecT[k][0:ksz, 128 * g:128 * g + tsz[g]],
                                     W[f"WihT_c_{k}"][0:ksz, :],
                                     start=(k == 0), stop=(k == 2))
                nc.vector.tensor_copy(XPc[g][:], xc[0:tsz[g], :])

        # ---- recurrence psum pools (persistent) ----
        psA = st.enter_context(tc.tile_pool(name="psA", bufs=1, space="PSUM"))
        ps_rzx_q = psA.tile([1, 512], F32, tag="rzx_q", name="rzx_q")
        ps_rzx_c = psA.tile([1, 512], F32, tag="rzx_c", name="rzx_c")
        ps_hn_qc = psA.tile([1, 512], F32, tag="hn_qc", name="hn_qc")
        ps_tr_qc = psA.tile([128, 4], F32, tag="tr_qc", name="tr_qc")

        def gru_step(g, j, XPt, msz, pos, HCols, ps_rzx, ps_hn, hnof, ps_tr, trof,
                     wih_pfx):
            r = rows[g]
            ca, cb = HCols[0][:, j:j + 1], HCols[1][:, j:j + 1]
            W0, W1 = W[f"WhhT_{wih_pfx}_0"], W[f"WhhT_{wih_pfx}_1"]
            # rz region
            nc.tensor.matmul(ps_rzx[0:1, 0:300], Ibf[0:msz, pos:pos + 1],
                             XPt[0:msz, 0:300], start=True, stop=False)
            nc.tensor.matmul(ps_rzx[0:1, 0:300], ca, W0[:, 0:300],
                             start=False, stop=False)
            nc.tensor.matmul(ps_rzx[0:1, 0:300], cb, W1[:, 0:300],
                             start=False, stop=True)
            # xn region
            nc.tensor.matmul(ps_rzx[0:1, 300:450], Ibf[0:msz, pos:pos + 1],
                             XPt[0:msz, 300:450], start=True, stop=True)
            # hn region
            nc.tensor.matmul(ps_hn[0:1, hnof:hnof + H], ca, W0[:, 300:450],
                             start=True, stop=False)
            nc.tensor.matmul(ps_hn[0:1, hnof:hnof + H], cb, W1[:, 300:450],
                             start=False, stop=True)
            # gates
            nc.scalar.activation(r["sig"][0:1, :], ps_rzx[0:1, 0:300], AF.Sigmoid)
            nc.vector.tensor_tensor(out=r["rhn"][0:1, :], in0=r["sig"][0:1, 0:H],
                                    in1=ps_hn[0:1, hnof:hnof + H], op=OP.mult)
            nc.vector.tensor_tensor(out=r["narg"][0:1, :], in0=r["rhn"][0:1, :],
                                    in1=ps_rzx[0:1, 300:450], op=OP.add)
            nc.scalar.activation(r["nn"][0:1, :], r["narg"][0:1, :], AF.Tanh)
            nc.vector.tensor_tensor(out=r["dd"][0:1, :], in0=r["hrow"][0:1, :],
                                    in1=r["nn"][0:1, :], op=OP.subtract)
            nc.vector.tensor_tensor(out=r["ee"][0:1, :], in0=r["sig"][0:1, H:300],
                                    in1=r["dd"][0:1, :], op=OP.mult)
            nc.vector.tensor_tensor(out=r["hrow"][0:1, :], in0=r["nn"][0:1, :],
                                    in1=r["ee"][0:1, :], op=OP.add)
            # transpose h2 -> columns (bf16)
            nc.tensor.transpose(ps_tr[0:128, trof:trof + 1],
                                r["hrow"][0:1, 0:128], Ifp[0:1, 0:1])
            nc.tensor.transpose(ps_tr[0:22, trof + 1:trof + 2],
                                r["hrow"][0:1, 128:150], Ifp[0:1, 0:1])
            nc.scalar.copy(HCols[0][:, j + 1:j + 2], ps_tr[0:128, trof:trof + 1])
            nc.scalar.copy(HCols[1][0:22, j + 1:j + 2],
                           ps_tr[0:22, trof + 1:trof + 2])

        for j in range(J):
            gru_step("q", j, XPq, J, j, HqC, ps_rzx_q, ps_hn_qc, 0,
                     ps_tr_qc, 0, "q")
        for t in range(T):
            g, pos = divmod(t, 128)
            gru_step("c", t, XPc[g], tsz[g], pos, HcC, ps_rzx_c, ps_hn_qc, 150,
                     ps_tr_qc, 2, "c")

        # ---- interlude: whq, HqR, ZX, WHP ----
        with tc.tile_pool(name="ps_i", bufs=1, space="PSUM") as psi:
            wq_ps = psi.tile([J, H], F32, tag="wq", name="wq")
            hq_ps = psi.tile([J, 128], BF16, tag="hqr", name="hqr")
            zx_ps = psi.tile([128, 450], F32, tag="zx", name="zx")
            wp_ps = psi.tile([128, H], F32, tag="wp", name="wp")
            nc.tensor.matmul(wq_ps[:], HqC[0][:, 1:J + 1], W["Wq_0"][:, :],
                             start=True, stop=False)
            nc.tensor.matmul(wq_ps[:], HqC[1][:, 1:J + 1], W["Wq_1"][:, :],
                             start=False, stop=True)
            nc.scalar.copy(whq[:], wq_ps[:])
            nc.tensor.transpose(hq_ps[0:J, 0:128], HqC[0][:, 1:J + 1],
                                Ibf[0:128, 0:128])
            nc.scalar.copy(HqR[:, 0:128], hq_ps[0:J, 0:128])
            nc.tensor.transpose(hq_ps[0:J, 0:22], HqC[1][0:22, 1:J + 1],
                                Ibf[0:22, 0:22])
            nc.scalar.copy(HqR[:, 128:150], hq_ps[0:J, 0:22])
            for g in range(NT):
                cs = slice(1 + 128 * g, 1 + 128 * g + tsz[g])
                nc.tensor.matmul(zx_ps[0:tsz[g], :], HcC[0][:, cs],
                                 W["WcT_0"][:, :], start=True, stop=False)
                nc.tensor.matmul(zx_ps[0:tsz[g], :], HcC[1][:, cs],
                                 W["WcT_1"][:, :], start=False, stop=True)
                nc.vector.tensor_copy(ZX[g][:], zx_ps[0:tsz[g], :])
                nc.tensor.matmul(wp_ps[0:tsz[g], :], HcC[0][:, cs],
                                 W["Wp_0"][:, :], start=True, stop=False)
                nc.tensor.matmul(wp_ps[0:tsz[g], :], HcC[1][:, cs],
                                 W["Wp_1"][:, :], start=False, stop=True)
                nc.vector.tensor_copy(WHP[g][:], wp_ps[0:tsz[g], :])

        # ---- match loop psums ----
        psM = st.enter_context(tc.tile_pool(name="psM", bufs=1, space="PSUM"))
        ps_rzx_m = psM.tile([1, 512], F32, tag="rzx_m", name="rzx_m")
        ps_hnm = psM.tile([1, 512], F32, tag="hnm", name="hnm")
        ps_trm = psM.tile([128, 4], F32, tag="trm", name="trm")
        ps_G = psM.tile([J, H], F32, tag="G", name="G")

        r = rows["m"]
        for t in range(T):
            g, pos = divmod(t, 128)
            # s = wh_r + wh_p_t  (psum [150:300] of ps_hnm)
            nc.tensor.matmul(ps_hnm[0:1, 150:300], hm[0][:, 0:1],
                             W["Wr_0"][:, :], start=True, stop=False)
            nc.tensor.matmul(ps_hnm[0:1, 150:300], hm[1][:, 0:1],
                             W["Wr_1"][:, :], start=False, stop=False)
            nc.tensor.matmul(ps_hnm[0:1, 150:300], Ibf[0:tsz[g], pos:pos + 1],
                             WHP[g][0:tsz[g], :], start=False, stop=True)
            nc.scalar.copy(s_row[0:1, :], ps_hnm[0:1, 150:300])
            # G = tanh(whq + ones x s)
            nc.tensor.matmul(ps_G[:], Ibf[0:J, 0:J], whq[:, :],
                             start=True, stop=False)
            nc.tensor.matmul(ps_G[:], ones64[0:2, :], s_row[0:2, :],
                             start=False, stop=True)
            nc.scalar.activation(G_sb[:], ps_G[:], AF.Tanh)
            # attn = sum_h G*w ; aq = Hq^T attn
            nc.vector.tensor_tensor(out=Gscr[:], in0=G_sb[:], in1=wbc[:],
                                    op=OP.mult)
            nc.vector.tensor_reduce(out=attn_f[:], in_=Gscr[:],
                                    axis=mybir.AxisListType.X, op=OP.add)
            nc.vector.tensor_copy(attn_b[:], attn_f[:])
            nc.tensor.matmul(ps_trm[0:128, 2:3], HqR[0:J, 0:128],
                             attn_b[0:J, 0:1], start=True, stop=True)
            nc.tensor.matmul(ps_trm[0:22, 3:4], HqR[0:J, 128:150],
                             attn_b[0:J, 0:1], start=True, stop=True)
            nc.scalar.copy(aq_a[:, 0:1], ps_trm[0:128, 2:3])
            nc.scalar.copy(aq_b[0:22, 0:1], ps_trm[0:22, 3:4])
            # gate projections
            nc.tensor.matmul(ps_rzx_m[0:1, 0:300], Ibf[0:tsz[g], pos:pos + 1],
                             ZX[g][0:tsz[g], 0:300], start=True, stop=False)
            nc.tensor.matmul(ps_rzx_m[0:1, 0:300], aq_a[:, 0:1],
                             W["W2T_0"][:, 0:300], start=False, stop=False)
            nc.tensor.matmul(ps_rzx_m[0:1, 0:300], aq_b[:, 0:1],
                             W["W2T_1"][:, 0:300], start=False, stop=False)
            nc.tensor.matmul(ps_rzx_m[0:1, 0:300], hm[0][:, 0:1],
                             W["WhhT_m_0"][:, 0:300], start=False, stop=False)
            nc.tensor.matmul(ps_rzx_m[0:1, 0:300], hm[1][:, 0:1],
                             W["WhhT_m_1"][:, 0:300], start=False, stop=True)
            nc.tensor.matmul(ps_rzx_m[0:1, 300:450], Ibf[0:tsz[g], pos:pos + 1],
                             ZX[g][0:tsz[g], 300:450], start=True, stop=False)
            nc.tensor.matmul(ps_rzx_m[0:1, 300:450], aq_a[:, 0:1],
                             W["W2T_0"][:, 300:450], start=False, stop=False)
            nc.tensor.matmul(ps_rzx_m[0:1, 300:450], aq_b[:, 0:1],
                             W["W2T_1"][:, 300:450], start=False, stop=True)
            nc.tensor.matmul(ps_hnm[0:1, 0:150], hm[0][:, 0:1],
                             W["WhhT_m_0"][:, 300:450], start=True, stop=False)
            nc.tensor.matmul(ps_hnm[0:1, 0:150], hm[1][:, 0:1],
                             W["WhhT_m_1"][:, 300:450], start=False, stop=True)
            # gates
            nc.scalar.activation(r["sig"][0:1, :], ps_rzx_m[0:1, 0:300], AF.Sigmoid)
            nc.vector.tensor_tensor(out=r["rhn"][0:1, :], in0=r["sig"][0:1, 0:H],
                                    in1=ps_hnm[0:1, 0:150], op=OP.mult)
            nc.vector.tensor_tensor(out=r["narg"][0:1, :], in0=r["rhn"][0:1, :],
                                    in1=ps_rzx_m[0:1, 300:450], op=OP.add)
            nc.scalar.activation(r["nn"][0:1, :], r["narg"][0:1, :], AF.Tanh)
            nc.vector.tensor_tensor(out=r["dd"][0:1, :], in0=r["hrow"][0:1, :],
                                    in1=r["nn"][0:1, :], op=OP.subtract)
            nc.vector.tensor_tensor(out=r["ee"][0:1, :], in0=r["sig"][0:1, H:300],
                                    in1=r["dd"][0:1, :], op=OP.mult)
            nc.vector.tensor_tensor(out=r["hrow"][0:1, :], in0=r["nn"][0:1, :],
                                    in1=r["ee"][0:1, :], op=OP.add)
            nc.tensor.transpose(ps_trm[0:128, 0:1], r["hrow"][0:1, 0:128],
                                Ifp[0:1, 0:1])
            nc.tensor.transpose(ps_trm[0:22, 1:2], r["hrow"][0:1, 128:150],
                                Ifp[0:1, 0:1])
            nc.scalar.copy(hm[0][:, 0:1], ps_trm[0:128, 0:1])
            nc.scalar.copy(hm[1][0:22, 0:1], ps_trm[0:22, 1:2])
            nc.sync.dma_start(hr_d.ap()[t + 1:t + 2, :], r["hrow"][0:1, :])

    nc.compile()
    return nc


def _bf(x):
    return np.ascontiguousarray(np.asarray(x, np.float32)).astype(BF)


def prep_shared(E, Wq, Wp, Wr, w, ctx_Wih, ctx_Whh, ctx_bih, ctx_bhh,
                q_Wih, q_Whh, q_bih, q_bhh, m_Wih, m_Whh, m_bih, m_bhh):
    f = {}
    f["Ifp"] = np.eye(128, dtype=np.float32)
    f["Ibf"] = _bf(np.eye(128))
    f["ones64"] = _bf(np.vstack([np.ones((1, 64)), np.zeros((1, 64))]))
    f["w_bcast"] = _bf(np.tile(np.asarray(w, np.float32)[None, :], (J, 1)))
    f["ones_row"] = _bf(np.ones((1, 512)))

    def wih_chunks(pfx, Wih, bih):
        WT = np.asarray(Wih, np.float32).T  # [300, 450]
        f[f"WihT_{pfx}_0"] = _bf(WT[0:128])
        f[f"WihT_{pfx}_1"] = _bf(WT[128:256])
        f[f"WihT_{pfx}_2"] = _bf(np.vstack([WT[256:300],
                                            np.asarray(bih, np.float32)[None, :]]))

    def whh_chunks(pfx, Whh, bhh):
        WT = np.asarray(Whh, np.float32).T  # [150, 450]
        f[f"WhhT_{pfx}_0"] = _bf(WT[0:128])
        f[f"WhhT_{pfx}_1"] = _bf(np.vstack([WT[128:150],
                                            np.asarray(bhh, np.float32)[None, :]]))

    def sq_chunks(pfx, M, brow=None):
        M = np.asarray(M, np.float32)  # [150, N]
        if brow is None:
            brow = np.zeros((1, M.shape[1]), np.float32)
        f[f"{pfx}_0"] = _bf(M[0:128])
        f[f"{pfx}_1"] = _bf(np.vstack([M[128:150], brow]))

    wih_chunks("q", q_Wih, q_bih)
    wih_chunks("c", ctx_Wih, ctx_bih)
    whh_chunks("q", q_Whh, q_bhh)
    whh_chunks("c", ctx_Whh, ctx_bhh)
    whh_chunks("m", m_Whh, m_bhh)
    m_Wih = np.asarray(m_Wih, np.float32)
    sq_chunks("W2T", m_Wih[:, H:].T, np.asarray(m_bih, np.float32)[None, :])
    sq_chunks("WcT", m_Wih[:, :H].T)
    sq_chunks("Wr", np.asarray(Wr, np.float32))
    sq_chunks("Wp", np.asarray(Wp, np.float32))
    sq_chunks("Wq", np.asarray(Wq, np.float32))
    return f


_NC_CACHE = {}


def kernel(context, query, E, Wq, Wp, Wr, w, ctx_Wih, ctx_Whh, ctx_bih,
           ctx_bhh, q_Wih, q_Whh, q_bih, q_bhh, m_Wih, m_Whh, m_bih, m_bhh,
           _T=None):
    context = np.asarray(context)
    query = np.asarray(query)
    B, T = context.shape
    if _T is not None:
        T = _T
        context = context[:, :T]
    NT = math.ceil(T / 128)
    if T not in _NC_CACHE:
        _NC_CACHE[T] = build(T)
    nc = _NC_CACHE[T]

    shared = prep_shared(E, Wq, Wp, Wr, w, ctx_Wih, ctx_Whh, ctx_bih, ctx_bhh,
                         q_Wih, q_Whh, q_bih, q_bhh, m_Wih, m_Whh, m_bih, m_bhh)
    E_np = np.ascontiguousarray(np.asarray(E, np.float32))
    in_maps = []
    for b in range(B):
        m = dict(shared)
        m["E"] = E_np
        ci = np.zeros((128, NT), np.int32)
        flat = np.asarray(context[b], np.int64).astype(np.int32)
        for g in range(NT):
            n = min(128, T - 128 * g)
            ci[0:n, g] = flat[128 * g:128 * g + n]
        m["ctx_idx"] = ci
        m["q_idx"] = np.asarray(query[b], np.int64).astype(np.int32)[:, None]
        in_maps.append(m)

    res = run_bass_kernel_spmd(nc, in_maps, core_ids=list(range(B)))
    out = np.stack([r["hr"] for r in res.results], axis=0)
    return out.astype(np.float32)



# revision 2
# speedup vs baseline: 1.0121x; 1.0121x over previous
"""MatchLSTM Trainium2 kernel v2: column-state recurrences, N=1 matmuls.

Per core (1 batch elem): embedding gather -> XP input projections (bulk) ->
interleaved q-GRU + ctx-GRU (64 rounds) -> whqT/HqW2 interlude ->
interleaved ctx-GRU + match recurrence (400 rounds, match lags 64) ->
bulk transpose of match-state history -> single output DMA.

State is kept as columns packed [75, 2] (h[0:75] | h[75:150]) so every
per-step matmul has out-free-size 1, and gate nonlinearities are per-
partition ACT/DVE ops. tensor_tensor_scan (free=1) fuses a*s+b.
"""
import math
from contextlib import ExitStack

import numpy as np
import ml_dtypes

import concourse.bacc as bacc
import concourse.bass as bass
import concourse.mybir as mybir
import concourse.tile as tile
from concourse.bass_utils import run_bass_kernel_spmd

F32 = mybir.dt.float32
BF16 = mybir.dt.bfloat16
I32 = mybir.dt.int32
AF = mybir.ActivationFunctionType
OP = mybir.AluOpType
BF = ml_dtypes.bfloat16

H = 150
HH = 75  # half hidden
D = 300
J = 64
V = 100000

# gate-half column ranges within the 450-wide gate dim: r0 r1 z0 z1 (n0 n1)
RZ_COLS = [(0, 75), (75, 150), (150, 225), (225, 300)]
N_COLS = [(300, 375), (375, 450)]

# packed-weight layout: name -> (partitions, cols); single DMA into one tile
W_SHAPES = [("Ibf", (128, 128))]
for _g in ("q", "c"):
    W_SHAPES += [(f"WihT_{_g}_0", (128, 450)), (f"WihT_{_g}_1", (128, 450)),
                 (f"WihT_{_g}_2", (45, 450))]
W_SPLIT_NAME = "WhhT75_q_0"  # everything before this is preamble-critical
for _g in ("q", "c", "m"):
    W_SHAPES += [(f"WhhT75_{_g}_0", (76, 450)), (f"WhhT75_{_g}_1", (75, 450))]
W_SHAPES += [("WcT75_0", (76, 450)), ("WcT75_1", (75, 450)),
             ("W2T75_0", (75, 450)), ("W2T75_1", (75, 450)),
             ("Wr75_0", (75, H)), ("Wr75_1", (75, H)),
             ("Wp75_0", (75, H)), ("Wp75_1", (75, H)),
             ("Wq75_0", (75, H)), ("Wq75_1", (75, H)),
             ("w75", (75, 2)), ("ones_bf", (1, 802)),
             ("WhhT75N_m_0", (75, 450)), ("WhhT75N_m_1", (75, 450)),
             ("WrN75_0", (75, H)), ("WrN75_1", (75, H)), ("zpad", (1, 2))]
W_OFF = {}
_c = 0
for _n, (_p, _w) in W_SHAPES:
    W_OFF[_n] = _c
    _c += _w
W_COLS = _c
W_SPLIT = W_OFF[W_SPLIT_NAME]


def _chunks(n, c=128):
    return [min(c, n - i) for i in range(0, n, c)]


def build(T=400):
    NT = math.ceil(T / 128)
    tsz = _chunks(T)

    nc = bacc.Bacc("TRN2", target_bir_lowering=False, debug=False, num_devices=8)

    dram = {}

    def din(name, shape, dt):
        dram[name] = nc.dram_tensor(name, list(shape), dt, kind="ExternalInput")
        return dram[name]

    E_d = din("E", [V, D], F32)
    din("ctx_idx", [128, NT], I32)
    din("q_idx", [J, 1], I32)
    din("Ifp", [128, 130], F32)   # Ifp [128,128] ++ ones_fp col pair
    din("W_all", [128, W_COLS], BF16)
    hr_d = nc.dram_tensor("hr", [T + 1, H], F32, kind="ExternalOutput")

    with tile.TileContext(nc) as tc, ExitStack() as st:
        sb = st.enter_context(tc.tile_pool(name="sb", bufs=1))

        def sbt(name, shape, dt):
            return sb.tile(list(shape), dt, tag=name, name=name)

        W_all = sbt("W_all", (128, W_COLS), BF16)
        W = {n: W_all[0:p, W_OFF[n]:W_OFF[n] + w] for n, (p, w) in W_SHAPES}
        Ibf = W["Ibf"]
        ones_bf = W["ones_bf"]
        IfpT = sbt("Ifp", (128, 130), F32)
        Ifp = IfpT[0:128, 0:128]
        ones_fp = IfpT[0:128, 128:130]
        cidx = sbt("cidx", (128, NT), I32)
        qidx = sbt("qidx", (J, 1), I32)
        ec = [sbt(f"ec{g}", (128, D), F32) for g in range(NT)]
        eq = sbt("eq", (J, D), F32)
        ecT = [sbt("ecT0", (128, T), BF16), sbt("ecT1", (128, T), BF16),
               sbt("ecT2", (45, T), BF16)]
        eqT = [sbt("eqT0", (128, J), BF16), sbt("eqT1", (128, J), BF16),
               sbt("eqT2", (45, J), BF16)]
        XPc = [sbt(f"XPc{g}", (tsz[g], 450), BF16) for g in range(NT)]
        XPq = sbt("XPq", (J, 450), BF16)
        # transposed n-gate input projections, cols = 2*t + half
        XPTn_c = sbt("XPTn_c", (HH, 2 * T), BF16)
        XPTn_q = sbt("XPTn_q", (HH, 2 * J), BF16)
        # state histories: [76, 2*(len+1)], row 75 == 1.0 (bias row)
        HqC = sbt("HqC", (76, 2 * (J + 1)), BF16)
        HcC = sbt("HcC", (76, 2 * (T + 1)), BF16)
        HmC = sbt("HmC", (76, 2 * (T + 1)), BF16)
        # fp32 carries (row 75 == 1.0 for the mixed-dtype rhs path)
        hqf = sbt("hqf", (76, 2), F32)
        hcf = sbt("hcf", (76, 2), F32)
        hmf = sbt("hmf", (76, 2), F32)
        # attention tiles
        whqT = sbt("whqT", (HH, 2 * J), BF16)   # cols 0:64 half0, 64:128 half1
        HqW2 = sbt("HqW2", (J, 450), BF16)
        GT = sbt("GT", (HH, 2 * J), BF16)
        GTarg = sbt("GTarg", (HH, 2 * J), BF16)
        s_sb = sbt("s_sb", (HH, 2), F32)
        attn_sb = sbt("attn_sb", (J, 1), BF16)
        hpn_sb = sbt("hpn_sb", (HH, 2), F32)
        # per-cell fp32 scratch
        nn = {g: sbt(f"nn_{g}", (HH, 2), F32) for g in ("q", "c", "m")}
        nn76 = sbt("nn76_m", (76, 2), BF16)   # row 75 = (1, 0) bias hook
        zh_m = sbt("zh_m", (HH, 2), BF16)
        zn_m = sbt("zn_m", (HH, 2), BF16)
        dd = {g: sbt(f"dd_{g}", (HH, 2), F32) for g in ("q", "c", "m")}
        tz = {g: sbt(f"tz_{g}", (HH, 2), F32) for g in ("q", "c", "m")}
        rsb = {g: sbt(f"rsb_{g}", (HH, 2), F32) for g in ("q", "c", "m")}
        xnsb = sbt("xnsb", (HH, 2), F32)
        # output staging
        HrS = sbt("HrS", (128, H), F32)

        # ---- persistent PSUM: 4 banks of [128, 512] fp32 ----
        psp = st.enter_context(tc.tile_pool(name="psp", bufs=1, space="PSUM"))
        psA = psp.tile([128, 512], F32, tag="psA", name="psA")
        psB = psp.tile([128, 512], F32, tag="psB", name="psB")
        psC = psp.tile([128, 512], F32, tag="psC", name="psC")
        psD = psp.tile([128, 512], F32, tag="psD", name="psD")
        psE = psp.tile([128, 1024], BF16, tag="psE", name="psE")
        par = [psA, psB]
        # region layout within a parity bank (columns):
        #   q gates 0:16, c gates 16:32, m gates 32:48, s 48:50, attn 50:51
        CELL_OFF = {"q": 0, "c": 16, "m": 32}
        # within a 16-col cell block: rz_in 0:4, hpn 4:6, xn 6:8, sig 8:12,
        # narg 12:14

        # ---- load constants / weights (batched; preamble-critical first) ----
        nc.sync.dma_start(cidx[:], dram["ctx_idx"].ap())
        nc.sync.dma_start(qidx[:], dram["q_idx"].ap())
        nc.sync.dma_start(IfpT[:], dram["Ifp"].ap())
        nc.sync.dma_start(W_all[0:128, 0:W_SPLIT],
                          dram["W_all"].ap()[0:128, 0:W_SPLIT])
        nc.sync.dma_start(W_all[0:128, W_SPLIT:W_COLS],
                          dram["W_all"].ap()[0:128, W_SPLIT:W_COLS])

        # ---- init state ----
        for hc, ncols in ((HqC, 2 * (J + 1)), (HcC, 2 * (T + 1)),
                          (HmC, 2 * (T + 1))):
            nc.vector.memset(hc[0:75, 0:2], 0.0)
            nc.sync.dma_start(
                hc[75:76, 0:ncols],
                dram["W_all"].ap()[0:1, W_OFF["ones_bf"]:W_OFF["ones_bf"] + ncols])
        for hf in (hqf, hcf, hmf):
            nc.vector.memset(hf[0:75, :], 0.0)
            nc.sync.dma_start(hf[75:76, 0:2], dram["Ifp"].ap()[0:1, 128:130])
        nc.vector.memset(nn76[0:75, :], 0.0)
        # row 75 = (1, 0): Ifp row 0 cols [128, 0] -> values (1.0, 0.0)
        nc.sync.dma_start(nn76[75:76, 0:1], dram["W_all"].ap()[0:1, W_OFF["ones_bf"]:W_OFF["ones_bf"] + 1])
        nc.sync.dma_start(nn76[75:76, 1:2], dram["W_all"].ap()[0:1, W_OFF["zpad"]:W_OFF["zpad"] + 1])
        nc.vector.memset(zh_m[:], 0.0)
        nc.vector.memset(zn_m[:], 0.0)

        # ---- gathers ----
        for g in range(NT):
            nc.gpsimd.indirect_dma_start(
                out=ec[g][:], out_offset=None, in_=E_d.ap(),
                in_offset=bass.IndirectOffsetOnAxis(ap=cidx[:, g:g + 1], axis=0))
        nc.gpsimd.indirect_dma_start(
            out=eq[:], out_offset=None, in_=E_d.ap(),
            in_offset=bass.IndirectOffsetOnAxis(ap=qidx[:, 0:1], axis=0))

        dch = [(0, 128), (128, 128), (256, 44)]

        # ---- preamble: q-side first; ctx-side deferred into phase 1 ----
        tri = 0
        for k, (doff, dsz) in enumerate(dch):
            c0 = 128 * (tri % 4)
            tri += 1
            tp = psC[0:dsz, c0:c0 + J]
            nc.tensor.transpose(tp, eq[0:J, doff:doff + dsz], Ifp[0:J, 0:J])
            nc.scalar.copy(eqT[k][0:dsz, 0:J], tp)
        ob = W_OFF["ones_bf"]
        nc.sync.dma_start(eqT[2][44:45, 0:J],
                          dram["W_all"].ap()[0:1, ob:ob + J])
        nc.sync.dma_start(ecT[2][44:45, 0:T],
                          dram["W_all"].ap()[0:1, ob:ob + T])
        xq = psA[0:J, 0:450]
        for k in range(3):
            ksz = [128, 128, 45][k]
            nc.tensor.matmul(xq, eqT[k][0:ksz, 0:J], W[f"WihT_q_{k}"][0:ksz, 0:450],
                             start=(k == 0), stop=(k == 2))
        nc.vector.tensor_copy(XPq[:], xq)
        pe_off = 800
        for half in range(2):
            c0, c1 = N_COLS[half]
            tp = psE[0:HH, pe_off:pe_off + J]
            pe_off += J
            nc.tensor.transpose(tp, XPq[0:J, c0:c1], Ibf[0:J, 0:J])
            nc.scalar.copy(XPTn_q[0:HH, half:2 * J:2], tp)

        def emit_ec_chunk(g):
            """ctx-side preamble for t-chunk g: transposes, XPc, XPTn_c."""
            nonlocal tri
            toff = 128 * g
            for k, (doff, dsz) in enumerate(dch):
                c0 = 128 * (tri % 4)
                tri += 1
                tp = psC[0:dsz, c0:c0 + tsz[g]]
                nc.tensor.transpose(tp, ec[g][0:tsz[g], doff:doff + dsz],
                                    Ifp[0:tsz[g], 0:tsz[g]])
                nc.scalar.copy(ecT[k][0:dsz, toff:toff + tsz[g]], tp)
            xc = (psB if g % 2 == 0 else psD)[0:tsz[g], 0:450]
            for k in range(3):
                ksz = [128, 128, 45][k]
                nc.tensor.matmul(xc[0:tsz[g], :],
                                 ecT[k][0:ksz, 128 * g:128 * g + tsz[g]],
                                 W[f"WihT_c_{k}"][0:ksz, 0:450],
                                 start=(k == 0), stop=(k == 2))
            nc.vector.tensor_copy(XPc[g][:], xc[0:tsz[g], :])
            for half in range(2):
                c0, c1 = N_COLS[half]
                tp = psE[0:HH, 200 * g + 100 * half:200 * g + 100 * half + tsz[g]]
                nc.tensor.transpose(tp, XPc[g][0:tsz[g], c0:c1],
                                    Ibf[0:tsz[g], 0:tsz[g]])
                nc.scalar.copy(
                    XPTn_c[0:HH, 2 * 128 * g + half:2 * (128 * g + tsz[g]):2],
                    tp)

        # ---- per-step emitters ----
        def gru_step(cell, t, XPt, msz, pos, HC, hf, XPTn):
            """One GRU step in column form. Reads state col pair t, writes
            pair t+1 and the fp32 carry."""
            ps = par[t % 2]
            o = CELL_OFF[cell]
            W0, W1 = W[f"WhhT75_{cell}_0"], W[f"WhhT75_{cell}_1"]
            r0, r1 = HC[0:76, 2 * t:2 * t + 1], HC[0:75, 2 * t + 1:2 * t + 2]
            # rz gates: psum cols o+0..o+3
            for mi, (m0, m1) in enumerate(RZ_COLS):
                pcol = ps[0:HH, o + mi:o + mi + 1]
                nc.tensor.matmul(pcol, XPt[0:msz, m0:m1],
                                 Ibf[0:msz, pos:pos + 1], start=True, stop=False)
                nc.tensor.matmul(pcol, W0[0:76, m0:m1], r0,
                                 start=False, stop=False)
                nc.tensor.matmul(pcol, W1[0:75, m0:m1], r1,
                                 start=False, stop=True)
            # hpn: psum cols o+4..o+5
            for half, (m0, m1) in enumerate(N_COLS):
                pcol = ps[0:HH, o + 4 + half:o + 5 + half]
                nc.tensor.matmul(pcol, W0[0:76, m0:m1], r0,
                                 start=True, stop=False)
                nc.tensor.matmul(pcol, W1[0:75, m0:m1], r1,
                                 start=False, stop=True)
            # r sigmoids to sbuf cols, then fused tanh(hpn*r + xn)
            for half in range(2):
                nc.scalar.activation(rsb[cell][0:HH, half:half + 1],
                                     ps[0:HH, o + half:o + half + 1],
                                     AF.Sigmoid)
            for half in range(2):
                nc.scalar.activation(
                    nn[cell][0:HH, half:half + 1],
                    ps[0:HH, o + 4 + half:o + 5 + half], AF.Tanh,
                    bias=XPTn[0:HH, 2 * t + half:2 * t + half + 1],
                    scale=rsb[cell][0:HH, half:half + 1])
            for half in range(2):
                nc.scalar.activation(ps[0:HH, o + 10 + half:o + 11 + half],
                                     ps[0:HH, o + 2 + half:o + 3 + half],
                                     AF.Sigmoid)
            # dd = h - n  (scan-sub per half)
            for half in range(2):
                nc.vector.tensor_tensor_scan(
                    out=dd[cell][0:HH, half:half + 1],
                    data0=HC[0:75, 2 * t + half:2 * t + half + 1],
                    data1=nn[cell][0:HH, half:half + 1],
                    initial=nn[cell][0:HH, half:half + 1],
                    op0=OP.subtract, op1=OP.bypass)
            # h2 = dd * z + n  (scan FMA, per half) -> bf16 history directly
            for half in range(2):
                nc.vector.tensor_tensor_scan(
                    out=HC[0:75, 2 * t + 2 + half:2 * t + 3 + half],
                    data0=dd[cell][0:HH, half:half + 1],
                    data1=nn[cell][0:HH, half:half + 1],
                    initial=ps[0:HH, o + 10 + half:o + 11 + half],
                    op0=OP.mult, op1=OP.add)


        def match_step(t, part=None):
            """One match-recurrence step. Uses ctx state col pair t+1.
            part='A' emits s/hpn/GT/attn; part='B' emits gates; None=both."""
            ps = par[t % 2]
            o = CELL_OFF["m"]
            g, pos = divmod(t, 128)
            XPt, msz = XPc[g], tsz[g]
            cc = 2 * (t + 1)  # ctx history col pair for hc_t
            r0 = HmC[0:76, 2 * t:2 * t + 1]
            r1 = HmC[0:75, 2 * t + 1:2 * t + 2]
            hc0, hc1 = HcC[0:75, cc:cc + 1], HcC[0:75, cc + 1:cc + 2]
            hc0b = HcC[0:76, cc:cc + 1]  # with bias row
            if part == "B":
                return _match_gates(t)
            # s = Wr @ hm + Wp @ hc : psum cols 48:50
            for half in range(2):
                pcol = ps[0:HH, 48 + half:49 + half]
                m0 = HH * half
                nc.tensor.matmul(pcol, W["Wr75_0"][0:75, m0:m0 + HH],
                                 HmC[0:75, 2 * t:2 * t + 1],
                                 start=True, stop=False)
                nc.tensor.matmul(pcol, W["Wr75_1"][0:75, m0:m0 + HH], r1,
                                 start=False, stop=False)
                nc.tensor.matmul(pcol, W["Wp75_0"][0:75, m0:m0 + HH], hc0,
                                 start=False, stop=False)
                nc.tensor.matmul(pcol, W["Wp75_1"][0:75, m0:m0 + HH], hc1,
                                 start=False, stop=True)
            # hpn: Whh_m n-cols (+bhh_n) : psum cols o+4..o+5
            for half, (m0, m1) in enumerate(N_COLS):
                pcol = ps[0:HH, o + 4 + half:o + 5 + half]
                nc.tensor.matmul(pcol, W["WhhT75_m_0"][0:76, m0:m1], r0,
                                 start=True, stop=False)
                nc.tensor.matmul(pcol, W["WhhT75_m_1"][0:75, m0:m1], r1,
                                 start=False, stop=True)
            # GT = tanh(whqT + s): DVE per-partition adds, one wide tanh
            for half in range(2):
                nc.vector.tensor_scalar_add(GTarg[0:HH, J * half:J * half + J],
                                            whqT[0:HH, J * half:J * half + J],
                                            ps[0:HH, 48 + half:49 + half])
            nc.scalar.activation(GT[0:HH, 0:2 * J], GTarg[0:HH, 0:2 * J],
                                 AF.Tanh)
            # attn = GT^T w : psum col 50 (rows 0:64)
            pat = ps[0:J, 50:51]
            nc.tensor.matmul(pat, GT[0:HH, 0:J], W["w75"][0:75, 0:1],
                             start=True, stop=False)
            nc.tensor.matmul(pat, GT[0:HH, J:2 * J], W["w75"][0:75, 1:2],
                             start=False, stop=True)
            nc.vector.tensor_copy(attn_sb[0:J, 0:1], pat)
            if part == "A":
                return
            # gates rz: zx (Wc, with bias row) + Whh_m + attn@HqW2
            for mi, (m0, m1) in enumerate(RZ_COLS):
                pcol = ps[0:HH, o + mi:o + mi + 1]
                nc.tensor.matmul(pcol, W["WcT75_0"][0:76, m0:m1], hc0b,
                                 start=True, stop=False)
                nc.tensor.matmul(pcol, W["WcT75_1"][0:75, m0:m1], hc1,
                                 start=False, stop=False)
                nc.tensor.matmul(pcol, W["WhhT75_m_0"][0:76, m0:m1], r0,
                                 start=False, stop=False)
                nc.tensor.matmul(pcol, W["WhhT75_m_1"][0:75, m0:m1], r1,
                                 start=False, stop=False)
                nc.tensor.matmul(pcol, HqW2[0:J, m0:m1], attn_sb[0:J, 0:1],
                                 start=False, stop=True)
            # xn: zx n-cols + attn@HqW2 n-cols : psum cols o+6..o+7
            for half, (m0, m1) in enumerate(N_COLS):
                pcol = ps[0:HH, o + 6 + half:o + 7 + half]
                nc.tensor.matmul(pcol, W["WcT75_0"][0:76, m0:m1], hc0b,
                                 start=True, stop=False)
                nc.tensor.matmul(pcol, W["WcT75_1"][0:75, m0:m1], hc1,
                                 start=False, stop=False)
                nc.tensor.matmul(pcol, HqW2[0:J, m0:m1], attn_sb[0:J, 0:1],
                                 start=False, stop=True)
            # xn to sbuf (free scan-copies), r sigmoids to sbuf,
            # fused tanh(hpn*r + xn) straight from the hpn psum
            for half in range(2):
                nc.vector.tensor_tensor_scan(
                    out=xnsb[0:HH, half:half + 1],
                    data0=ps[0:HH, o + 6 + half:o + 7 + half],
                    data1=W["w75"][0:HH, 0:1],
                    initial=0.0, op0=OP.bypass, op1=OP.bypass)
            for half in range(2):
                nc.scalar.activation(rsb["m"][0:HH, half:half + 1],
                                     ps[0:HH, o + half:o + half + 1],
                                     AF.Sigmoid)
            for half in range(2):
                nc.scalar.activation(
                    nn["m"][0:HH, half:half + 1],
                    ps[0:HH, o + 4 + half:o + 5 + half], AF.Tanh,
                    bias=xnsb[0:HH, half:half + 1],
                    scale=rsb["m"][0:HH, half:half + 1])
            for half in range(2):
                nc.scalar.activation(ps[0:HH, o + 10 + half:o + 11 + half],
                                     ps[0:HH, o + 2 + half:o + 3 + half],
                                     AF.Sigmoid)
            for half in range(2):
                nc.vector.tensor_tensor_scan(
                    out=dd["m"][0:HH, half:half + 1],
                    data0=HmC[0:75, 2 * t + half:2 * t + half + 1],
                    data1=nn["m"][0:HH, half:half + 1],
                    initial=nn["m"][0:HH, half:half + 1],
                    op0=OP.subtract, op1=OP.bypass)
            for half in range(2):
                nc.vector.tensor_tensor_scan(
                    out=HmC[0:75, 2 * t + 2 + half:2 * t + 3 + half],
                    data0=dd["m"][0:HH, half:half + 1],
                    data1=nn["m"][0:HH, half:half + 1],
                    initial=ps[0:HH, o + 10 + half:o + 11 + half],
                    op0=OP.mult, op1=OP.add)

        # ---- phase 1: q-GRU || ctx-GRU (rounds 0..63) ----
        for j in range(J):
            if j < NT:
                emit_ec_chunk(j)
            gru_step("q", j, XPq, J, j, HqC, hqf, XPTn_q)
            g, pos = divmod(j, 128)
            gru_step("c", j, XPc[g], tsz[g], pos, HcC, hcf, XPTn_c)

        # ---- interlude: whqT, HqW2 ----
        # whqT[p, 64*hb + j] = sum_h Wq[h, 75*hb + p] * Hq[j, h]
        pw = psC[0:HH, 0:128]
        for hb in range(2):
            for k in range(2):
                nc.tensor.matmul(
                    pw[0:HH, J * hb:J * hb + J],
                    W[f"Wq75_{k}"][0:75, HH * hb:HH * hb + HH],
                    HqC[0:75, 2 + k:2 * (J + 1):2],
                    start=(k == 0), stop=(k == 1))
        nc.vector.tensor_copy(whqT[:], pw)
        pq = psD[0:J, 0:450]
        for k in range(2):
            nc.tensor.matmul(pq, HqC[0:75, 2 + k:2 * (J + 1):2],
                             W[f"W2T75_{k}"][0:75, 0:450],
                             start=(k == 0), stop=(k == 1))
        nc.vector.tensor_copy(HqW2[:], pq)

        # ---- phase 2: ctx-GRU || match (rounds 64..T+63) ----
        for r in range(J, T + J):
            match_step(r - J)
            if r < T:
                g, pos = divmod(r, 128)
                gru_step("c", r, XPc[g], tsz[g], pos, HcC, hcf, XPTn_c)

        # ---- output: transpose HmC history -> rows -> DRAM ----
        rch = _chunks(T + 1)
        for ci, csz in enumerate(rch):
            c0 = 128 * ci
            for half in range(2):
                tp = psE[0:csz, 256 + 80 * half:256 + 80 * half + HH]
                nc.tensor.transpose(
                    tp, HmC[0:75, 2 * c0 + half:min(2 * (c0 + csz) + half, 2 * (T + 1)):2],
                    Ibf[0:75, 0:75])
                nc.scalar.copy(HrS[0:csz, HH * half:HH * half + HH], tp)
            nc.sync.dma_start(hr_d.ap()[c0:c0 + csz, :], HrS[0:csz, :])

    nc.compile()
    return nc


def _bf(x):
    return np.ascontiguousarray(np.asarray(x, np.float32)).astype(BF)


def prep_shared(E, Wq, Wp, Wr, w, ctx_Wih, ctx_Whh, ctx_bih, ctx_bhh,
                q_Wih, q_Whh, q_bih, q_bhh, m_Wih, m_Whh, m_bih, m_bhh):
    f = {}
    ifp = np.zeros((128, 130), np.float32)
    ifp[:, 0:128] = np.eye(128, dtype=np.float32)
    ifp[:, 128:130] = 1.0
    f["Ifp"] = ifp

    w8 = {}
    w8["Ibf"] = _bf(np.eye(128))
    w8["ones_bf"] = _bf(np.ones((1, 802)))
    w8["w75"] = _bf(np.asarray(w, np.float32).reshape(2, 75).T)

    def wih_chunks(pfx, Wih, bih, bhh):
        WT = np.asarray(Wih, np.float32).T  # [300, 450]
        bias = np.asarray(bih, np.float32).copy()
        bias[:300] += np.asarray(bhh, np.float32)[:300]  # bhh_rz folded
        w8[f"WihT_{pfx}_0"] = _bf(WT[0:128])
        w8[f"WihT_{pfx}_1"] = _bf(WT[128:256])
        w8[f"WihT_{pfx}_2"] = _bf(np.vstack([WT[256:300], bias[None, :]]))

    def whh_chunks(pfx, Whh, bhh):
        WT = np.asarray(Whh, np.float32).T  # [150, 450]
        brow = np.zeros((1, 450), np.float32)
        brow[0, 300:450] = np.asarray(bhh, np.float32)[300:450]  # bhh_n
        w8[f"WhhT75_{pfx}_0"] = _bf(np.vstack([WT[0:75], brow]))
        w8[f"WhhT75_{pfx}_1"] = _bf(WT[75:150])

    wih_chunks("q", q_Wih, q_bih, q_bhh)
    wih_chunks("c", ctx_Wih, ctx_bih, ctx_bhh)
    whh_chunks("q", q_Whh, q_bhh)
    whh_chunks("c", ctx_Whh, ctx_bhh)
    whh_chunks("m", m_Whh, m_bhh)

    m_Wih = np.asarray(m_Wih, np.float32)
    WcT = m_Wih[:, :H].T  # [150, 450]
    brow = np.asarray(m_bih, np.float32).copy()
    brow[:300] += np.asarray(m_bhh, np.float32)[:300]
    w8["WcT75_0"] = _bf(np.vstack([WcT[0:75], brow[None, :]]))
    w8["WcT75_1"] = _bf(WcT[75:150])
    W2T = m_Wih[:, H:].T  # [150, 450]
    w8["W2T75_0"] = _bf(W2T[0:75])
    w8["W2T75_1"] = _bf(W2T[75:150])
    for pfx, M in (("Wr", Wr), ("Wp", Wp), ("Wq", Wq)):
        M = np.asarray(M, np.float32)
        w8[f"{pfx}75_0"] = _bf(M[0:75])
        w8[f"{pfx}75_1"] = _bf(M[75:150])

    WhhmT = np.asarray(m_Whh, np.float32).T
    w8["WhhT75N_m_0"] = _bf(-WhhmT[0:75])
    w8["WhhT75N_m_1"] = _bf(-WhhmT[75:150])
    WrF = np.asarray(Wr, np.float32)
    w8["WrN75_0"] = _bf(-WrF[0:75])
    w8["WrN75_1"] = _bf(-WrF[75:150])
    w8["zpad"] = _bf(np.zeros((1, 2)))
    wall = np.zeros((128, W_COLS), BF)
    for n, (p, wcols) in W_SHAPES:
        wall[0:p, W_OFF[n]:W_OFF[n] + wcols] = w8[n]
    f["W_all"] = wall
    return f


_NC_CACHE = {}


def kernel(context, query, E, Wq, Wp, Wr, w, ctx_Wih, ctx_Whh, ctx_bih,
           ctx_bhh, q_Wih, q_Whh, q_bih, q_bhh, m_Wih, m_Whh, m_bih, m_bhh,
           _T=None):
    context = np.asarray(context)
    query = np.asarray(query)
    B, T = context.shape
    if _T is not None:
        T = _T
        context = context[:, :T]
    NT = math.ceil(T / 128)
    if T not in _NC_CACHE:
        _NC_CACHE[T] = build(T)
    nc = _NC_CACHE[T]

    shared = prep_shared(E, Wq, Wp, Wr, w, ctx_Wih, ctx_Whh, ctx_bih, ctx_bhh,
                         q_Wih, q_Whh, q_bih, q_bhh, m_Wih, m_Whh, m_bih,
                         m_bhh)
    E_np = np.ascontiguousarray(np.asarray(E, np.float32))
    in_maps = []
    for b in range(B):
        m = dict(shared)
        m["E"] = E_np
        ci = np.zeros((128, NT), np.int32)
        flat = np.asarray(context[b], np.int64).astype(np.int32)
        for g in range(NT):
            n = min(128, T - 128 * g)
            ci[0:n, g] = flat[128 * g:128 * g + n]
        m["ctx_idx"] = ci
        m["q_idx"] = np.asarray(query[b], np.int64).astype(np.int32)[:, None]
        in_maps.append(m)

    res = run_bass_kernel_spmd(nc, in_maps, core_ids=list(range(B)))
    out = np.stack([r["hr"] for r in res.results], axis=0)
    return out.astype(np.float32)


# revision 3
# speedup vs baseline: 1.0123x; 1.0002x over previous
"""MatchLSTM Trainium2 kernel v2: column-state recurrences, N=1 matmuls.

Per core (1 batch elem): embedding gather -> XP input projections (bulk) ->
interleaved q-GRU + ctx-GRU (64 rounds) -> whqT/HqW2 interlude ->
interleaved ctx-GRU + match recurrence (400 rounds, match lags 64) ->
bulk transpose of match-state history -> single output DMA.

State is kept as columns packed [75, 2] (h[0:75] | h[75:150]) so every
per-step matmul has out-free-size 1, and gate nonlinearities are per-
partition ACT/DVE ops. tensor_tensor_scan (free=1) fuses a*s+b.
"""
import math
from contextlib import ExitStack

import numpy as np
import ml_dtypes

import concourse.bacc as bacc
import concourse.bass as bass
import concourse.mybir as mybir
import concourse.tile as tile
from concourse.bass_utils import run_bass_kernel_spmd

F32 = mybir.dt.float32
BF16 = mybir.dt.bfloat16
I32 = mybir.dt.int32
AF = mybir.ActivationFunctionType
OP = mybir.AluOpType
BF = ml_dtypes.bfloat16

H = 150
HH = 75  # half hidden
D = 300
J = 64
V = 100000

# gate-half column ranges within the 450-wide gate dim: r0 r1 z0 z1 (n0 n1)
RZ_COLS = [(0, 75), (75, 150), (150, 225), (225, 300)]
N_COLS = [(300, 375), (375, 450)]

# packed-weight layout: name -> (partitions, cols); single DMA into one tile
W_SHAPES = [("Ibf", (128, 128))]
for _g in ("q", "c"):
    W_SHAPES += [(f"WihT_{_g}_0", (128, 450)), (f"WihT_{_g}_1", (128, 450)),
                 (f"WihT_{_g}_2", (45, 450))]
W_SPLIT_NAME = "WhhT75_q_0"  # everything before this is preamble-critical
for _g in ("q", "c", "m"):
    W_SHAPES += [(f"WhhT75_{_g}_0", (76, 450)), (f"WhhT75_{_g}_1", (75, 450))]
W_SHAPES += [("WcT75_0", (76, 450)), ("WcT75_1", (75, 450)),
             ("W2T75_0", (75, 450)), ("W2T75_1", (75, 450)),
             ("Wr75_0", (75, H)), ("Wr75_1", (75, H)),
             ("Wp75_0", (75, H)), ("Wp75_1", (75, H)),
             ("Wq75_0", (75, H)), ("Wq75_1", (75, H)),
             ("w75", (75, 2)), ("ones_bf", (1, 802)),
             ("WhhT75N_m_0", (75, 450)), ("WhhT75N_m_1", (75, 450)),
             ("WrN75_0", (75, H)), ("WrN75_1", (75, H)), ("zpad", (1, 2))]
W_OFF = {}
_c = 0
for _n, (_p, _w) in W_SHAPES:
    W_OFF[_n] = _c
    _c += _w
W_COLS = _c
W_SPLIT = W_OFF[W_SPLIT_NAME]


def _chunks(n, c=128):
    return [min(c, n - i) for i in range(0, n, c)]


def build(T=400):
    NT = math.ceil(T / 128)
    tsz = _chunks(T)

    nc = bacc.Bacc("TRN2", target_bir_lowering=False, debug=False, num_devices=8)

    dram = {}

    def din(name, shape, dt):
        dram[name] = nc.dram_tensor(name, list(shape), dt, kind="ExternalInput")
        return dram[name]

    E_d = din("E", [V, D], F32)
    din("ctx_idx", [128, NT], I32)
    din("q_idx", [J, 1], I32)
    din("Ifp", [128, 130], F32)   # Ifp [128,128] ++ ones_fp col pair
    din("W_all", [128, W_COLS], BF16)
    hr_d = nc.dram_tensor("hr", [T + 1, H], F32, kind="ExternalOutput")

    with tile.TileContext(nc) as tc, ExitStack() as st:
        sb = st.enter_context(tc.tile_pool(name="sb", bufs=1))

        def sbt(name, shape, dt):
            return sb.tile(list(shape), dt, tag=name, name=name)

        W_all = sbt("W_all", (128, W_COLS), BF16)
        W = {n: W_all[0:p, W_OFF[n]:W_OFF[n] + w] for n, (p, w) in W_SHAPES}
        Ibf = W["Ibf"]
        ones_bf = W["ones_bf"]
        IfpT = sbt("Ifp", (128, 130), F32)
        Ifp = IfpT[0:128, 0:128]
        ones_fp = IfpT[0:128, 128:130]
        cidx = sbt("cidx", (128, NT), I32)
        qidx = sbt("qidx", (J, 1), I32)
        ec = [sbt(f"ec{g}", (128, D), F32) for g in range(NT)]
        eq = sbt("eq", (J, D), F32)
        ecT = [sbt("ecT0", (128, T), BF16), sbt("ecT1", (128, T), BF16),
               sbt("ecT2", (45, T), BF16)]
        eqT = [sbt("eqT0", (128, J), BF16), sbt("eqT1", (128, J), BF16),
               sbt("eqT2", (45, J), BF16)]
        XPc = [sbt(f"XPc{g}", (tsz[g], 450), BF16) for g in range(NT)]
        XPq = sbt("XPq", (J, 450), BF16)
        # transposed n-gate input projections, cols = 2*t + half
        XPTn_c = sbt("XPTn_c", (HH, 2 * T), BF16)
        XPTn_q = sbt("XPTn_q", (HH, 2 * J), BF16)
        # state histories: [76, 2*(len+1)], row 75 == 1.0 (bias row)
        HqC = sbt("HqC", (76, 2 * (J + 1)), BF16)
        HcC = sbt("HcC", (76, 2 * (T + 1)), BF16)
        HmC = sbt("HmC", (76, 2 * (T + 1)), BF16)
        # fp32 carries (row 75 == 1.0 for the mixed-dtype rhs path)
        hqf = sbt("hqf", (76, 2), F32)
        hcf = sbt("hcf", (76, 2), F32)
        hmf = sbt("hmf", (76, 2), F32)
        # attention tiles
        whqT = sbt("whqT", (HH, 2 * J), BF16)   # cols 0:64 half0, 64:128 half1
        HqW2 = sbt("HqW2", (J, 450), BF16)
        GT = sbt("GT", (HH, 2 * J), BF16)
        GTarg = sbt("GTarg", (HH, 2 * J), BF16)
        s_sb = sbt("s_sb", (HH, 2), F32)
        attn_sb = sbt("attn_sb", (J, 1), BF16)
        hpn_sb = sbt("hpn_sb", (HH, 2), F32)
        # per-cell fp32 scratch
        nn = {g: sbt(f"nn_{g}", (HH, 2), F32) for g in ("q", "c", "m")}
        nn76 = sbt("nn76_m", (76, 2), BF16)   # row 75 = (1, 0) bias hook
        zh_m = sbt("zh_m", (HH, 2), BF16)
        zn_m = sbt("zn_m", (HH, 2), BF16)
        dd = {g: sbt(f"dd_{g}", (HH, 2), F32) for g in ("q", "c", "m")}
        tz = {g: sbt(f"tz_{g}", (HH, 2), F32) for g in ("q", "c", "m")}
        rsb = {g: sbt(f"rsb_{g}", (HH, 2), F32) for g in ("q", "c", "m")}
        xnsb = sbt("xnsb", (HH, 2), F32)
        # output staging
        HrS = sbt("HrS", (128, H), F32)

        # ---- persistent PSUM: 4 banks of [128, 512] fp32 ----
        psp = st.enter_context(tc.tile_pool(name="psp", bufs=1, space="PSUM"))
        psA = psp.tile([128, 512], F32, tag="psA", name="psA")
        psB = psp.tile([128, 512], F32, tag="psB", name="psB")
        psC = psp.tile([128, 512], F32, tag="psC", name="psC")
        psD = psp.tile([128, 512], F32, tag="psD", name="psD")
        psE = psp.tile([128, 1024], BF16, tag="psE", name="psE")
        par = [psA, psB]
        # region layout within a parity bank (columns):
        #   q gates 0:16, c gates 16:32, m gates 32:48, s 48:50, attn 50:51
        CELL_OFF = {"q": 0, "c": 16, "m": 32}
        # within a 16-col cell block: rz_in 0:4, hpn 4:6, xn 6:8, sig 8:12,
        # narg 12:14

        # ---- load constants / weights (batched; preamble-critical first) ----
        nc.sync.dma_start(cidx[:], dram["ctx_idx"].ap())
        nc.sync.dma_start(qidx[:], dram["q_idx"].ap())
        nc.sync.dma_start(IfpT[:], dram["Ifp"].ap())
        nc.sync.dma_start(W_all[0:128, 0:W_SPLIT],
                          dram["W_all"].ap()[0:128, 0:W_SPLIT])
        nc.sync.dma_start(W_all[0:128, W_SPLIT:W_COLS],
                          dram["W_all"].ap()[0:128, W_SPLIT:W_COLS])

        # ---- init state ----
        for hc, ncols in ((HqC, 2 * (J + 1)), (HcC, 2 * (T + 1)),
                          (HmC, 2 * (T + 1))):
            nc.vector.memset(hc[0:75, 0:2], 0.0)
            nc.sync.dma_start(
                hc[75:76, 0:ncols],
                dram["W_all"].ap()[0:1, W_OFF["ones_bf"]:W_OFF["ones_bf"] + ncols])
        for hf in (hqf, hcf, hmf):
            nc.vector.memset(hf[0:75, :], 0.0)
            nc.sync.dma_start(hf[75:76, 0:2], dram["Ifp"].ap()[0:1, 128:130])
        nc.vector.memset(nn76[0:75, :], 0.0)
        # row 75 = (1, 0): Ifp row 0 cols [128, 0] -> values (1.0, 0.0)
        nc.sync.dma_start(nn76[75:76, 0:1], dram["W_all"].ap()[0:1, W_OFF["ones_bf"]:W_OFF["ones_bf"] + 1])
        nc.sync.dma_start(nn76[75:76, 1:2], dram["W_all"].ap()[0:1, W_OFF["zpad"]:W_OFF["zpad"] + 1])
        nc.vector.memset(zh_m[:], 0.0)
        nc.vector.memset(zn_m[:], 0.0)

        # ---- gathers (q first: it opens the recurrence pipeline) ----
        nc.gpsimd.indirect_dma_start(
            out=eq[:], out_offset=None, in_=E_d.ap(),
            in_offset=bass.IndirectOffsetOnAxis(ap=qidx[:, 0:1], axis=0))
        for g in range(NT):
            nc.gpsimd.indirect_dma_start(
                out=ec[g][:], out_offset=None, in_=E_d.ap(),
                in_offset=bass.IndirectOffsetOnAxis(ap=cidx[:, g:g + 1], axis=0))

        dch = [(0, 128), (128, 128), (256, 44)]

        # ---- preamble: q-side first; ctx-side deferred into phase 1 ----
        tri = 0
        for k, (doff, dsz) in enumerate(dch):
            c0 = 128 * (tri % 4)
            tri += 1
            tp = psC[0:dsz, c0:c0 + J]
            nc.tensor.transpose(tp, eq[0:J, doff:doff + dsz], Ifp[0:J, 0:J])
            nc.scalar.copy(eqT[k][0:dsz, 0:J], tp)
        ob = W_OFF["ones_bf"]
        nc.sync.dma_start(eqT[2][44:45, 0:J],
                          dram["W_all"].ap()[0:1, ob:ob + J])
        nc.sync.dma_start(ecT[2][44:45, 0:T],
                          dram["W_all"].ap()[0:1, ob:ob + T])
        xq = psA[0:J, 0:450]
        for k in range(3):
            ksz = [128, 128, 45][k]
            nc.tensor.matmul(xq, eqT[k][0:ksz, 0:J], W[f"WihT_q_{k}"][0:ksz, 0:450],
                             start=(k == 0), stop=(k == 2))
        nc.vector.tensor_copy(XPq[:], xq)
        pe_off = 800
        for half in range(2):
            c0, c1 = N_COLS[half]
            tp = psE[0:HH, pe_off:pe_off + J]
            pe_off += J
            nc.tensor.transpose(tp, XPq[0:J, c0:c1], Ibf[0:J, 0:J])
            nc.scalar.copy(XPTn_q[0:HH, half:2 * J:2], tp)

        def emit_ec_chunk(g):
            """ctx-side preamble for t-chunk g: transposes, XPc, XPTn_c."""
            nonlocal tri
            toff = 128 * g
            for k, (doff, dsz) in enumerate(dch):
                c0 = 128 * (tri % 4)
                tri += 1
                tp = psC[0:dsz, c0:c0 + tsz[g]]
                nc.tensor.transpose(tp, ec[g][0:tsz[g], doff:doff + dsz],
                                    Ifp[0:tsz[g], 0:tsz[g]])
                nc.scalar.copy(ecT[k][0:dsz, toff:toff + tsz[g]], tp)
            xc = (psB if g % 2 == 0 else psD)[0:tsz[g], 0:450]
            for k in range(3):
                ksz = [128, 128, 45][k]
                nc.tensor.matmul(xc[0:tsz[g], :],
                                 ecT[k][0:ksz, 128 * g:128 * g + tsz[g]],
                                 W[f"WihT_c_{k}"][0:ksz, 0:450],
                                 start=(k == 0), stop=(k == 2))
            nc.vector.tensor_copy(XPc[g][:], xc[0:tsz[g], :])
            for half in range(2):
                c0, c1 = N_COLS[half]
                tp = psE[0:HH, 200 * g + 100 * half:200 * g + 100 * half + tsz[g]]
                nc.tensor.transpose(tp, XPc[g][0:tsz[g], c0:c1],
                                    Ibf[0:tsz[g], 0:tsz[g]])
                nc.scalar.copy(
                    XPTn_c[0:HH, 2 * 128 * g + half:2 * (128 * g + tsz[g]):2],
                    tp)

        # ---- per-step emitters ----
        def gru_step(cell, t, XPt, msz, pos, HC, hf, XPTn):
            """One GRU step in column form. Reads state col pair t, writes
            pair t+1 and the fp32 carry."""
            ps = par[t % 2]
            o = CELL_OFF[cell]
            W0, W1 = W[f"WhhT75_{cell}_0"], W[f"WhhT75_{cell}_1"]
            r0, r1 = HC[0:76, 2 * t:2 * t + 1], HC[0:75, 2 * t + 1:2 * t + 2]
            # rz gates: psum cols o+0..o+3
            for mi, (m0, m1) in enumerate(RZ_COLS):
                pcol = ps[0:HH, o + mi:o + mi + 1]
                nc.tensor.matmul(pcol, XPt[0:msz, m0:m1],
                                 Ibf[0:msz, pos:pos + 1], start=True, stop=False)
                nc.tensor.matmul(pcol, W0[0:76, m0:m1], r0,
                                 start=False, stop=False)
                nc.tensor.matmul(pcol, W1[0:75, m0:m1], r1,
                                 start=False, stop=True)
            # hpn: psum cols o+4..o+5
            for half, (m0, m1) in enumerate(N_COLS):
                pcol = ps[0:HH, o + 4 + half:o + 5 + half]
                nc.tensor.matmul(pcol, W0[0:76, m0:m1], r0,
                                 start=True, stop=False)
                nc.tensor.matmul(pcol, W1[0:75, m0:m1], r1,
                                 start=False, stop=True)
            # r sigmoids to sbuf cols, then fused tanh(hpn*r + xn)
            for half in range(2):
                nc.scalar.activation(rsb[cell][0:HH, half:half + 1],
                                     ps[0:HH, o + half:o + half + 1],
                                     AF.Sigmoid)
            for half in range(2):
                nc.scalar.activation(
                    nn[cell][0:HH, half:half + 1],
                    ps[0:HH, o + 4 + half:o + 5 + half], AF.Tanh,
                    bias=XPTn[0:HH, 2 * t + half:2 * t + half + 1],
                    scale=rsb[cell][0:HH, half:half + 1])
            for half in range(2):
                nc.scalar.activation(ps[0:HH, o + 10 + half:o + 11 + half],
                                     ps[0:HH, o + 2 + half:o + 3 + half],
                                     AF.Sigmoid)
            # dd = h - n  (scan-sub per half)
            for half in range(2):
                nc.vector.tensor_tensor_scan(
                    out=dd[cell][0:HH, half:half + 1],
                    data0=HC[0:75, 2 * t + half:2 * t + half + 1],
                    data1=nn[cell][0:HH, half:half + 1],
                    initial=nn[cell][0:HH, half:half + 1],
                    op0=OP.subtract, op1=OP.bypass)
            # h2 = dd * z + n  (scan FMA, per half) -> bf16 history directly
            for half in range(2):
                nc.vector.tensor_tensor_scan(
                    out=HC[0:75, 2 * t + 2 + half:2 * t + 3 + half],
                    data0=dd[cell][0:HH, half:half + 1],
                    data1=nn[cell][0:HH, half:half + 1],
                    initial=ps[0:HH, o + 10 + half:o + 11 + half],
                    op0=OP.mult, op1=OP.add)


        def match_step(t, part=None):
            """One match-recurrence step. Uses ctx state col pair t+1.
            part='A' emits s/hpn/GT/attn; part='B' emits gates; None=both."""
            ps = par[t % 2]
            o = CELL_OFF["m"]
            g, pos = divmod(t, 128)
            XPt, msz = XPc[g], tsz[g]
            cc = 2 * (t + 1)  # ctx history col pair for hc_t
            r0 = HmC[0:76, 2 * t:2 * t + 1]
            r1 = HmC[0:75, 2 * t + 1:2 * t + 2]
            hc0, hc1 = HcC[0:75, cc:cc + 1], HcC[0:75, cc + 1:cc + 2]
            hc0b = HcC[0:76, cc:cc + 1]  # with bias row
            if part == "B":
                return _match_gates(t)
            # s = Wr @ hm + Wp @ hc : psum cols 48:50
            for half in range(2):
                pcol = ps[0:HH, 48 + half:49 + half]
                m0 = HH * half
                nc.tensor.matmul(pcol, W["Wr75_0"][0:75, m0:m0 + HH],
                                 HmC[0:75, 2 * t:2 * t + 1],
                                 start=True, stop=False)
                nc.tensor.matmul(pcol, W["Wr75_1"][0:75, m0:m0 + HH], r1,
                                 start=False, stop=False)
                nc.tensor.matmul(pcol, W["Wp75_0"][0:75, m0:m0 + HH], hc0,
                                 start=False, stop=False)
                nc.tensor.matmul(pcol, W["Wp75_1"][0:75, m0:m0 + HH], hc1,
                                 start=False, stop=True)
            # hpn: Whh_m n-cols (+bhh_n) : psum cols o+4..o+5
            for half, (m0, m1) in enumerate(N_COLS):
                pcol = ps[0:HH, o + 4 + half:o + 5 + half]
                nc.tensor.matmul(pcol, W["WhhT75_m_0"][0:76, m0:m1], r0,
                                 start=True, stop=False)
                nc.tensor.matmul(pcol, W["WhhT75_m_1"][0:75, m0:m1], r1,
                                 start=False, stop=True)
            # GT = tanh(whqT + s): DVE per-partition adds, one wide tanh
            for half in range(2):
                nc.vector.tensor_scalar_add(GTarg[0:HH, J * half:J * half + J],
                                            whqT[0:HH, J * half:J * half + J],
                                            ps[0:HH, 48 + half:49 + half])
            nc.scalar.activation(GT[0:HH, 0:2 * J], GTarg[0:HH, 0:2 * J],
                                 AF.Tanh)
            # attn = GT^T w : psum col 50 (rows 0:64)
            pat = ps[0:J, 50:51]
            nc.tensor.matmul(pat, GT[0:HH, 0:J], W["w75"][0:75, 0:1],
                             start=True, stop=False)
            nc.tensor.matmul(pat, GT[0:HH, J:2 * J], W["w75"][0:75, 1:2],
                             start=False, stop=True)
            nc.vector.tensor_copy(attn_sb[0:J, 0:1], pat)
            if part == "A":
                return
            # gates rz: zx (Wc, with bias row) + Whh_m + attn@HqW2
            for mi, (m0, m1) in enumerate(RZ_COLS):
                pcol = ps[0:HH, o + mi:o + mi + 1]
                nc.tensor.matmul(pcol, W["WcT75_0"][0:76, m0:m1], hc0b,
                                 start=True, stop=False)
                nc.tensor.matmul(pcol, W["WcT75_1"][0:75, m0:m1], hc1,
                                 start=False, stop=False)
                nc.tensor.matmul(pcol, W["WhhT75_m_0"][0:76, m0:m1], r0,
                                 start=False, stop=False)
                nc.tensor.matmul(pcol, W["WhhT75_m_1"][0:75, m0:m1], r1,
                                 start=False, stop=False)
                nc.tensor.matmul(pcol, HqW2[0:J, m0:m1], attn_sb[0:J, 0:1],
                                 start=False, stop=True)
            # xn: zx n-cols + attn@HqW2 n-cols : psum cols o+6..o+7
            for half, (m0, m1) in enumerate(N_COLS):
                pcol = ps[0:HH, o + 6 + half:o + 7 + half]
                nc.tensor.matmul(pcol, W["WcT75_0"][0:76, m0:m1], hc0b,
                                 start=True, stop=False)
                nc.tensor.matmul(pcol, W["WcT75_1"][0:75, m0:m1], hc1,
                                 start=False, stop=False)
                nc.tensor.matmul(pcol, HqW2[0:J, m0:m1], attn_sb[0:J, 0:1],
                                 start=False, stop=True)
            # xn to sbuf (free scan-copies), r sigmoids to sbuf,
            # fused tanh(hpn*r + xn) straight from the hpn psum
            for half in range(2):
                nc.vector.tensor_tensor_scan(
                    out=xnsb[0:HH, half:half + 1],
                    data0=ps[0:HH, o + 6 + half:o + 7 + half],
                    data1=W["w75"][0:HH, 0:1],
                    initial=0.0, op0=OP.bypass, op1=OP.bypass)
            for half in range(2):
                nc.scalar.activation(rsb["m"][0:HH, half:half + 1],
                                     ps[0:HH, o + half:o + half + 1],
                                     AF.Sigmoid)
            for half in range(2):
                nc.scalar.activation(
                    nn["m"][0:HH, half:half + 1],
                    ps[0:HH, o + 4 + half:o + 5 + half], AF.Tanh,
                    bias=xnsb[0:HH, half:half + 1],
                    scale=rsb["m"][0:HH, half:half + 1])
            for half in range(2):
                nc.scalar.activation(ps[0:HH, o + 10 + half:o + 11 + half],
                                     ps[0:HH, o + 2 + half:o + 3 + half],
                                     AF.Sigmoid)
            for half in range(2):
                nc.vector.tensor_tensor_scan(
                    out=dd["m"][0:HH, half:half + 1],
                    data0=HmC[0:75, 2 * t + half:2 * t + half + 1],
                    data1=nn["m"][0:HH, half:half + 1],
                    initial=nn["m"][0:HH, half:half + 1],
                    op0=OP.subtract, op1=OP.bypass)
            for half in range(2):
                nc.vector.tensor_tensor_scan(
                    out=HmC[0:75, 2 * t + 2 + half:2 * t + 3 + half],
                    data0=dd["m"][0:HH, half:half + 1],
                    data1=nn["m"][0:HH, half:half + 1],
                    initial=ps[0:HH, o + 10 + half:o + 11 + half],
                    op0=OP.mult, op1=OP.add)

        # ---- phase 1: q-GRU || ctx-GRU (rounds 0..63) ----
        for j in range(J):
            if j < NT:
                emit_ec_chunk(j)
            gru_step("q", j, XPq, J, j, HqC, hqf, XPTn_q)
            g, pos = divmod(j, 128)
            gru_step("c", j, XPc[g], tsz[g], pos, HcC, hcf, XPTn_c)

        # ---- interlude: whqT, HqW2 ----
        # whqT[p, 64*hb + j] = sum_h Wq[h, 75*hb + p] * Hq[j, h]
        pw = psC[0:HH, 0:128]
        for hb in range(2):
            for k in range(2):
                nc.tensor.matmul(
                    pw[0:HH, J * hb:J * hb + J],
                    W[f"Wq75_{k}"][0:75, HH * hb:HH * hb + HH],
                    HqC[0:75, 2 + k:2 * (J + 1):2],
                    start=(k == 0), stop=(k == 1))
        nc.vector.tensor_copy(whqT[:], pw)
        pq = psD[0:J, 0:450]
        for k in range(2):
            nc.tensor.matmul(pq, HqC[0:75, 2 + k:2 * (J + 1):2],
                             W[f"W2T75_{k}"][0:75, 0:450],
                             start=(k == 0), stop=(k == 1))
        nc.vector.tensor_copy(HqW2[:], pq)

        # ---- phase 2: ctx-GRU || match (rounds 64..T+63) ----
        for r in range(J, T + J):
            match_step(r - J)
            if r < T:
                g, pos = divmod(r, 128)
                gru_step("c", r, XPc[g], tsz[g], pos, HcC, hcf, XPTn_c)

        # ---- output: transpose HmC history -> rows -> DRAM ----
        rch = _chunks(T + 1)
        for ci, csz in enumerate(rch):
            c0 = 128 * ci
            for half in range(2):
                tp = psE[0:csz, 256 + 80 * half:256 + 80 * half + HH]
                nc.tensor.transpose(
                    tp, HmC[0:75, 2 * c0 + half:min(2 * (c0 + csz) + half, 2 * (T + 1)):2],
                    Ibf[0:75, 0:75])
                nc.scalar.copy(HrS[0:csz, HH * half:HH * half + HH], tp)
            nc.sync.dma_start(hr_d.ap()[c0:c0 + csz, :], HrS[0:csz, :])

    nc.compile()
    return nc


def _bf(x):
    return np.ascontiguousarray(np.asarray(x, np.float32)).astype(BF)


def prep_shared(E, Wq, Wp, Wr, w, ctx_Wih, ctx_Whh, ctx_bih, ctx_bhh,
                q_Wih, q_Whh, q_bih, q_bhh, m_Wih, m_Whh, m_bih, m_bhh):
    f = {}
    ifp = np.zeros((128, 130), np.float32)
    ifp[:, 0:128] = np.eye(128, dtype=np.float32)
    ifp[:, 128:130] = 1.0
    f["Ifp"] = ifp

    w8 = {}
    w8["Ibf"] = _bf(np.eye(128))
    w8["ones_bf"] = _bf(np.ones((1, 802)))
    w8["w75"] = _bf(np.asarray(w, np.float32).reshape(2, 75).T)

    def wih_chunks(pfx, Wih, bih, bhh):
        WT = np.asarray(Wih, np.float32).T  # [300, 450]
        bias = np.asarray(bih, np.float32).copy()
        bias[:300] += np.asarray(bhh, np.float32)[:300]  # bhh_rz folded
        w8[f"WihT_{pfx}_0"] = _bf(WT[0:128])
        w8[f"WihT_{pfx}_1"] = _bf(WT[128:256])
        w8[f"WihT_{pfx}_2"] = _bf(np.vstack([WT[256:300], bias[None, :]]))

    def whh_chunks(pfx, Whh, bhh):
        WT = np.asarray(Whh, np.float32).T  # [150, 450]
        brow = np.zeros((1, 450), np.float32)
        brow[0, 300:450] = np.asarray(bhh, np.float32)[300:450]  # bhh_n
        w8[f"WhhT75_{pfx}_0"] = _bf(np.vstack([WT[0:75], brow]))
        w8[f"WhhT75_{pfx}_1"] = _bf(WT[75:150])

    wih_chunks("q", q_Wih, q_bih, q_bhh)
    wih_chunks("c", ctx_Wih, ctx_bih, ctx_bhh)
    whh_chunks("q", q_Whh, q_bhh)
    whh_chunks("c", ctx_Whh, ctx_bhh)
    whh_chunks("m", m_Whh, m_bhh)

    m_Wih = np.asarray(m_Wih, np.float32)
    WcT = m_Wih[:, :H].T  # [150, 450]
    brow = np.asarray(m_bih, np.float32).copy()
    brow[:300] += np.asarray(m_bhh, np.float32)[:300]
    w8["WcT75_0"] = _bf(np.vstack([WcT[0:75], brow[None, :]]))
    w8["WcT75_1"] = _bf(WcT[75:150])
    W2T = m_Wih[:, H:].T  # [150, 450]
    w8["W2T75_0"] = _bf(W2T[0:75])
    w8["W2T75_1"] = _bf(W2T[75:150])
    for pfx, M in (("Wr", Wr), ("Wp", Wp), ("Wq", Wq)):
        M = np.asarray(M, np.float32)
        w8[f"{pfx}75_0"] = _bf(M[0:75])
        w8[f"{pfx}75_1"] = _bf(M[75:150])

    WhhmT = np.asarray(m_Whh, np.float32).T
    w8["WhhT75N_m_0"] = _bf(-WhhmT[0:75])
    w8["WhhT75N_m_1"] = _bf(-WhhmT[75:150])
    WrF = np.asarray(Wr, np.float32)
    w8["WrN75_0"] = _bf(-WrF[0:75])
    w8["WrN75_1"] = _bf(-WrF[75:150])
    w8["zpad"] = _bf(np.zeros((1, 2)))
    wall = np.zeros((128, W_COLS), BF)
    for n, (p, wcols) in W_SHAPES:
        wall[0:p, W_OFF[n]:W_OFF[n] + wcols] = w8[n]
    f["W_all"] = wall
    return f


_NC_CACHE = {}


def kernel(context, query, E, Wq, Wp, Wr, w, ctx_Wih, ctx_Whh, ctx_bih,
           ctx_bhh, q_Wih, q_Whh, q_bih, q_bhh, m_Wih, m_Whh, m_bih, m_bhh,
           _T=None):
    context = np.asarray(context)
    query = np.asarray(query)
    B, T = context.shape
    if _T is not None:
        T = _T
        context = context[:, :T]
    NT = math.ceil(T / 128)
    if T not in _NC_CACHE:
        _NC_CACHE[T] = build(T)
    nc = _NC_CACHE[T]

    shared = prep_shared(E, Wq, Wp, Wr, w, ctx_Wih, ctx_Whh, ctx_bih, ctx_bhh,
                         q_Wih, q_Whh, q_bih, q_bhh, m_Wih, m_Whh, m_bih,
                         m_bhh)
    E_np = np.ascontiguousarray(np.asarray(E, np.float32))
    in_maps = []
    for b in range(B):
        m = dict(shared)
        m["E"] = E_np
        ci = np.zeros((128, NT), np.int32)
        flat = np.asarray(context[b], np.int64).astype(np.int32)
        for g in range(NT):
            n = min(128, T - 128 * g)
            ci[0:n, g] = flat[128 * g:128 * g + n]
        m["ctx_idx"] = ci
        m["q_idx"] = np.asarray(query[b], np.int64).astype(np.int32)[:, None]
        in_maps.append(m)

    res = run_bass_kernel_spmd(nc, in_maps, core_ids=list(range(B)))
    out = np.stack([r["hr"] for r in res.results], axis=0)
    return out.astype(np.float32)
